# revision 6
# baseline (speedup 1.0000x reference)
"""MeanAggregator v2: bucketed dma_gather + TensorE matmul regroup.

out[n, :] = mean_k features[neigh_idx[n, k], :]   (N=100000, K=6,
V=200000, D=128, f32).  8 cores, nodes sharded 12500/core.

Per core:
  - entries (n, k) bucketed by table chunk (7 chunks of 28572 rows so
    the chunk-local row index fits int16 for dma_gather); node->tile
    assignment balanced per core so every (tile, chunk) fits one
    128-slot unit.
  - per (node-tile t, chunk c): ceil(count/128) 128-slot units; unit slots
    hold chunk-local row indices (pad idx 0).  dma_gather (ucode mlp lib)
    fetches each unit into one 128-partition staging column (bf16, rows
    pre-scaled by 1/6 on host): slot s -> partition s%128.
  - per tile: DVE is_equal(nid, iota) builds 0/1 selection matrices B^T
    [slot, node] (pad slots nid=255 -> all-zero column), then TensorE
    accumulates sum_units B^T.T @ unit into PSUM [node, feat] = the mean.
  - PSUM -> SBUF copy (Act engine), grouped 8-tile output DMAs.

Measured on 8 axon trn2 cores: ~235 us device time (neuron-profile,
max over cores; baseline indirect-DMA version: ~843 us), rel err ~2.1e-3
vs the f32 reference (bf16 feature quantization).  Key limits: SWDGE
descriptor generation ~2.0 us per 1024-idx dma_gather instruction
(994 ns fixed + ~1 ns/desc, Pool-engine serial, 91 instructions) with
the HBM random-read drain (~20.5 ns/desc/engine) and PE matmuls
(~290 ns each, low p-state) overlapped underneath.

Staging columns are laid out chunk-major (so each <=8-unit gather
instruction writes consecutive columns); nid columns are laid out
tile-major (so each tile's B build reads consecutive columns).  The unit
structure depends on the indices, so the Bass program is built (and
compiled) per problem instance; SPMD across cores uses the max unit count
per (t, c) over the 8 cores (cores pad unused slots: idx 0, nid 255).
"""

import numpy as np
import ml_dtypes

import concourse.bass as bass
import concourse.bacc as bacc
import concourse.mybir as mybir
import concourse.tile as tile
from concourse import library_config
from concourse.bass_utils import run_bass_kernel_spmd

N = 100000
K = 6
V = 200000
D = 128
NCORES = 8
P = 128

NPC = N // NCORES            # 12500 nodes per core
TILES = -(-NPC // P)         # 98 node tiles per core
NPAD = TILES * P             # 12544
NCHUNK = 7
CH = -(-V // NCHUNK)         # 28572 rows per chunk (< 32768 for int16)
SB = 13                      # sub-batches (stages) per core: 7-8 tiles each,
                             # so each (sub-batch, chunk) is one <=1024-idx
                             # gather instruction and PE work bursts densely
MAXU = 8                     # units (128 idx each) per gather instruction
GROUP = 8                    # tiles per output DMA (>= tiles per sub-batch)
PADNID = 255.0
# feature dtype: bf16 on HW; f32 for CoreSim validation (sim lacks bf16
# dma_gather support)
FDT_M = mybir.dt.bfloat16
FDT_NP = ml_dtypes.bfloat16


def _split_tiles(tiles, sb):
    base, rem = divmod(tiles, sb)
    out, t0 = [], 0
    for s in range(sb):
        n = base + (1 if s < rem else 0)
        out.append((t0, t0 + n))
        t0 += n
    return out


def _balance(sig: np.ndarray, cap: int = P) -> np.ndarray | None:
    """Assign NPAD padded nodes (rows of sig: per-chunk entry counts; pads
    all-zero) to TILES groups of 128 so every per-group per-chunk count
    <= cap.  Returns pos[NPAD] (node -> slot; tile = pos//128) or None.

    Natural order start + swap repair: the expected overflow is a few
    dozen (t, c) cells a few entries over, with ~16% global slack.
    """
    npad, nch = sig.shape
    node_at = np.arange(npad)                 # slot -> node
    loads = sig.reshape(TILES, P, nch).sum(axis=1)    # [TILES, nch]
    for _ in range(4000):
        over = np.argwhere(loads > cap)
        if len(over) == 0:
            pos = np.empty(npad, np.int64)
            pos[node_at] = np.arange(npad)
            return pos
        g, c = over[np.argmax(loads[over[:, 0], over[:, 1]])]
        slots_g = np.arange(g * P, (g + 1) * P)
        sg = sig[node_at[slots_g]]            # [128, nch]
        n_local = int(np.argmax(sg[:, c]))
        n_slot = g * P + n_local
        sn = sig[node_at[n_slot]]
        # allowed post-swap load for g: strict progress on c, no new
        # overload beyond current level elsewhere
        glim = np.maximum(cap, loads[g])
        glim[c] = loads[g, c] - 1
        ok = False
        for t in np.argsort(loads[:, c]):
            if t == g:
                continue
            slots_t = np.arange(t * P, (t + 1) * P)
            st = sig[node_at[slots_t]]        # [128, nch]
            newg = loads[g][None, :] - sn[None, :] + st
            newt = loads[t][None, :] + sn[None, :] - st
            feas = ((newt <= cap).all(axis=1) & (newg <= glim).all(axis=1))
            if feas.any():
                cand = np.where(feas)[0]
                m_local = int(cand[np.argmin(st[cand, c])])
                m_slot = t * P + m_local
                sm = sig[node_at[m_slot]]
                loads[g] += sm - sn
                loads[t] += sn - sm
                node_at[n_slot], node_at[m_slot] = (
                    node_at[m_slot], node_at[n_slot])
                ok = True
                break
        if not ok:
            return None
    return None


class Plan:
    """Shared (SPMD) unit structure + per-core host arrays."""

    def __init__(self, neigh_idx: np.ndarray):
        ni = np.asarray(neigh_idx).astype(np.int64).reshape(NCORES, NPC, K)
        counts = np.zeros((NCORES, TILES, NCHUNK), np.int64)
        ents = []
        # pos of padded original node -> slot (tile = slot//128); balanced so
        # per-(tile, chunk) counts stay <= 128 (falls back to natural order
        # + multi-column units if repair fails)
        self.pos = np.zeros((NCORES, NPAD), np.int64)
        for core in range(NCORES):
            idx = ni[core].reshape(-1)              # NPC*K entries
            cc = idx // CH
            sig = np.zeros((NPAD, NCHUNK), np.int64)
            np.add.at(sig, (np.repeat(np.arange(NPC), K), cc), 1)
            pos = _balance(sig)
            if pos is None:
                pos = np.arange(NPAD)
            self.pos[core] = pos
            nn = np.repeat(pos[:NPC], K)            # slot of each entry
            tt = nn // P
            ll = (idx - cc * CH).astype(np.int16)
            key = tt * NCHUNK + cc
            counts[core] = np.bincount(
                key, minlength=TILES * NCHUNK).reshape(TILES, NCHUNK)
            ents.append((nn, tt, cc, ll, key))
        self.cols = -(-counts.max(axis=0) // P)      # [TILES, NCHUNK]
        assert (self.cols.sum(axis=1) > 0).all()
        self.maxj = int(self.cols.max())

        self.sb_ranges = _split_tiles(TILES, SB)
        self.nsb = len(self.sb_ranges)
        # chunk-major staging cols + tile-major nid cols, per sub-batch
        self.unit_col = {}       # (t,c,j) -> staging col within sb
        self.unit_tm = {}        # (t,c,j) -> tile-major col within sb
        self.tile_tmbase = {}    # t -> first tile-major col
        self.sb_ncols = []
        self.instr = [[] for _ in range(self.nsb)]    # (c, u0, take)
        self.idx_off = [[] for _ in range(self.nsb)]
        self.sb_idxcols = []
        for s, (ta, tb) in enumerate(self.sb_ranges):
            col = 0
            icols = 0
            for c in range(NCHUNK):
                cstart = col
                for t in range(ta, tb):
                    for j in range(self.cols[t, c]):
                        self.unit_col[(t, c, j)] = col
                        col += 1
                nu = col - cstart
                u0 = cstart
                while nu > 0:
                    take = min(MAXU, nu)
                    self.instr[s].append((c, u0, take))
                    self.idx_off[s].append(icols)
                    icols += take * P // 16
                    u0 += take
                    nu -= take
            tm = 0
            for t in range(ta, tb):
                self.tile_tmbase[t] = tm
                for c in range(NCHUNK):
                    for j in range(self.cols[t, c]):
                        self.unit_tm[(t, c, j)] = tm
                        tm += 1
            assert tm == col
            self.sb_ncols.append(col)
            self.sb_idxcols.append(icols)
        self.total_units = sum(self.sb_ncols)
        self.sb_colbase = np.concatenate(
            [[0], np.cumsum(self.sb_ncols)]).astype(np.int64)
        self.sb_idxbase = np.concatenate(
            [[0], np.cumsum(self.sb_idxcols)]).astype(np.int64)

        self.idx_dram = np.zeros((NCORES, P, int(self.sb_idxbase[-1])),
                                 np.int16)
        nid_f32 = np.full((NCORES, P, self.total_units), PADNID, np.float32)

        sbmap = np.zeros((TILES,), np.int64)
        for s, (ta, tb) in enumerate(self.sb_ranges):
            sbmap[ta:tb] = s
        MJ = self.maxj
        colmap = np.full((TILES * NCHUNK * MJ,), -1, np.int64)
        tmmap = np.full((TILES * NCHUNK * MJ,), -1, np.int64)
        for (t, c, j), col in self.unit_col.items():
            colmap[(t * NCHUNK + c) * MJ + j] = col
            tmmap[(t * NCHUNK + c) * MJ + j] = self.unit_tm[(t, c, j)]
        # per sb: staging col -> (idx dram col base, pos within instr)
        ucolbase = [np.zeros(n, np.int64) for n in self.sb_ncols]
        upos = [np.zeros(n, np.int64) for n in self.sb_ncols]
        for s in range(self.nsb):
            for (c, u0, take), ioff in zip(self.instr[s], self.idx_off[s]):
                for u in range(u0, u0 + take):
                    ucolbase[s][u] = self.sb_idxbase[s] + ioff
                    upos[s][u] = u - u0

        for core in range(NCORES):
            nn, tt, cc, ll, key = ents[core]
            # secondary sort by chunk-local row index: each unit's 128
            # descriptors read ascending HBM addresses
            order = np.lexsort((ll, key))
            skey = key[order]
            group_start = np.searchsorted(skey, np.arange(TILES * NCHUNK))
            rank = np.arange(len(order)) - group_start[skey]
            jj = rank // P
            pp = rank % P
            snn, stt, scc, sll = nn[order], tt[order], cc[order], ll[order]
            tcj = (stt * NCHUNK + scc) * MJ + jj
            ecol = colmap[tcj]
            etm = tmmap[tcj]
            assert (ecol >= 0).all()
            esb = sbmap[stt]
            gtm = self.sb_colbase[esb] + etm
            nid_f32[core, pp, gtm] = (snn % P).astype(np.float32)
            for s in range(self.nsb):
                m = esb == s
                if not m.any():
                    continue
                fl = upos[s][ecol[m]] * P + pp[m]
                icol = ucolbase[s][ecol[m]] + fl // 16
                irow = fl % 16
                for g in range(8):
                    self.idx_dram[core, irow + 16 * g, icol] = sll[m]
        self.nid_dram = nid_f32.astype(FDT_NP)

    def signature(self):
        return tuple(self.cols.reshape(-1).tolist())


def build_nc(plan: Plan, nq=4):
    nc = bacc.Bacc("TRN2", target_bir_lowering=False, num_swdge_queues=nq)
    featb = nc.dram_tensor("featb", [V, D], FDT_M,
                           kind="ExternalInput")
    idx = nc.dram_tensor("idx", [P, int(plan.sb_idxbase[-1])],
                         mybir.dt.int16, kind="ExternalInput")
    nid = nc.dram_tensor("nid", [P, int(plan.total_units)],
                         FDT_M, kind="ExternalInput")
    iota = nc.dram_tensor("iota", [P, P], FDT_M,
                          kind="ExternalInput")
    out = nc.dram_tensor("out", [P, TILES * D], mybir.dt.float32,
                         kind="ExternalOutput")
    with tile.TileContext(nc) as tc:
        nc.gpsimd.load_library(library_config.mlp)
        with tc.tile_pool(name="const", bufs=1) as constp, \
             tc.tile_pool(name="idxp", bufs=4) as idxp, \
             tc.tile_pool(name="nidp", bufs=4) as nidp, \
             tc.tile_pool(name="stg", bufs=4) as stgp, \
             tc.tile_pool(name="bp", bufs=3) as bp, \
             tc.tile_pool(name="accp", bufs=3) as accp, \
             tc.psum_pool(name="wps", bufs=1) as wpsp, \
             tc.psum_pool(name="ps", bufs=4) as psp:
            iota_t = constp.tile([P, P], FDT_M)
            nc.sync.dma_start(out=iota_t[:], in_=iota[:])
            # prologue warmers: an early dma_gather (fed straight from the
            # first idx DMA, no DVE dependency) absorbs the Q7 cold-start
            # (~8us: library reload + icache) before the first real gather;
            # dummy matmuls keep the PE p-state up while gathers fill the
            # first sub-batches.
            dstg = constp.tile([P, D], FDT_M)
            didx = constp.tile([P, 8], mybir.dt.int16)
            nc.gpsimd.memset(didx[:], 0)
            nc.gpsimd.dma_gather(
                out_ap=dstg[:].rearrange("p (j d) -> p j d", j=1, d=D),
                in_ap=featb[0:CH, :], idxs_ap=didx[:],
                num_idxs=P, num_idxs_reg=P, elem_size=D)
            wps = wpsp.tile([P, P], mybir.dt.float32)
            for _ in range(64):
                nc.tensor.matmul(wps[:], lhsT=iota_t[:], rhs=iota_t[:],
                                 start=True, stop=True)
            qi = 0
            for s in range(plan.nsb):
                ta, tb = plan.sb_ranges[s]
                ic0 = int(plan.sb_idxbase[s])
                icn = int(plan.sb_idxcols[s])
                cb = int(plan.sb_colbase[s])
                ncols = int(plan.sb_ncols[s])
                idx_t = idxp.tile([P, icn], mybir.dt.int16)
                nc.sync.dma_start(out=idx_t[:], in_=idx[:, ic0:ic0 + icn])
                nid_t = nidp.tile([P, ncols], FDT_M)
                nc.sync.dma_start(out=nid_t[:], in_=nid[:, cb:cb + ncols])
                stg = stgp.tile([P, ncols * D], FDT_M)
                for (c, u0, take), ioff in zip(plan.instr[s],
                                               plan.idx_off[s]):
                    ni_i = take * P
                    nc.gpsimd.dma_gather(
                        out_ap=stg[:, u0 * D:(u0 + take) * D].rearrange(
                            "p (u d) -> p u d", u=take, d=D),
                        in_ap=featb[c * CH:min((c + 1) * CH, V), :],
                        idxs_ap=idx_t[:, ioff:ioff + ni_i // 16],
                        num_idxs=ni_i,
                        num_idxs_reg=ni_i,
                        elem_size=D,
                        queue_num=qi % nq,
                    )
                    qi += 1
                acc = None
                for t in range(ta, tb):
                    ucols = [plan.unit_col[(t, c, j)]
                             for c in range(NCHUNK)
                             for j in range(plan.cols[t, c])]
                    nu = len(ucols)
                    tmb = plan.tile_tmbase[t]
                    gi_t = (t - ta) % GROUP
                    if gi_t == 0:
                        gsz = min(GROUP, tb - t)
                        acc = accp.tile([P, gsz * D], mybir.dt.float32)
                    bt = bp.tile([P, nu * P], FDT_M)
                    in0 = nid_t[:, tmb:tmb + nu].rearrange(
                        "p (u one) -> p u one", u=nu, one=1)
                    in1 = iota_t[:].rearrange(
                        "p (one d) -> p one d", one=1, d=P)
                    b0, b1 = bass.broadcast_tensor_aps(in0, in1)
                    nc.vector.tensor_tensor(
                        out=bt[:].rearrange("p (u d) -> p u d", u=nu, d=P),
                        in0=b0, in1=b1, op=mybir.AluOpType.is_equal)
                    ps = psp.tile([P, D], mybir.dt.float32)
                    for ui, ucol in enumerate(ucols):
                        nc.tensor.matmul(
                            ps[:],
                            lhsT=bt[:, ui * P:(ui + 1) * P],
                            rhs=stg[:, ucol * D:(ucol + 1) * D],
                            start=(ui == 0),
                            stop=(ui == nu - 1),
                        )
                    nc.scalar.copy(
                        out=acc[:, gi_t * D:(gi_t + 1) * D], in_=ps[:])
                    if gi_t == GROUP - 1 or t == tb - 1:
                        t0 = t - gi_t
                        nc.sync.dma_start(
                            out=out[:, t0 * D:(t + 1) * D],
                            in_=acc[:, :(gi_t + 1) * D])
    nc.compile()
    return nc


_cache = {}


def _get(plan):
    sig = plan.signature()
    if sig not in _cache:
        _cache[sig] = build_nc(plan)
    return _cache[sig]


def make_in_maps(features, neigh_idx, plan):
    feat = np.asarray(features, dtype=np.float32) * np.float32(1.0 / K)
    featb = feat.astype(FDT_NP)
    iota = np.ascontiguousarray(np.broadcast_to(
        np.arange(P, dtype=np.float32), (P, P))).astype(FDT_NP)
    return [{"featb": featb, "idx": np.ascontiguousarray(plan.idx_dram[c]),
             "nid": np.ascontiguousarray(plan.nid_dram[c]),
             "iota": np.ascontiguousarray(iota)}
            for c in range(NCORES)]


def assemble_out(results, plan):
    outs = []
    for c in range(NCORES):
        o = results[c]["out"]
        o = o.reshape(P, TILES, D).transpose(1, 0, 2).reshape(NPAD, D)
        outs.append(o[plan.pos[c, :NPC]])
    return np.ascontiguousarray(np.concatenate(outs, axis=0))


def kernel(features: np.ndarray, neigh_idx: np.ndarray, **run_kwargs):
    plan = Plan(neigh_idx)
    in_maps = make_in_maps(features, neigh_idx, plan)
    res = run_bass_kernel_spmd(_get(plan), in_maps,
                               core_ids=list(range(NCORES)), **run_kwargs)
    full = assemble_out(res.results, plan)
    if run_kwargs:
        return full, res
    return full


# revision 7
# speedup vs baseline: 1.0306x; 1.0306x over previous
"""MeanAggregator v2: bucketed dma_gather + TensorE matmul regroup.

out[n, :] = mean_k features[neigh_idx[n, k], :]   (N=100000, K=6,
V=200000, D=128, f32).  8 cores, nodes sharded 12500/core.

Per core:
  - entries (n, k) bucketed by table chunk (7 chunks of 28572 rows so
    the chunk-local row index fits int16 for dma_gather); node->tile
    assignment balanced per core so every (tile, chunk) fits one
    128-slot unit.
  - per (node-tile t, chunk c): ceil(count/128) 128-slot units; unit slots
    hold chunk-local row indices (pad idx 0).  dma_gather (ucode mlp lib)
    fetches each unit into one 128-partition staging column (bf16, rows
    pre-scaled by 1/6 on host): slot s -> partition s%128.
  - per tile: DVE is_equal(nid, iota) builds 0/1 selection matrices B^T
    [slot, node] (pad slots nid=255 -> all-zero column), then TensorE
    accumulates sum_units B^T.T @ unit into PSUM [node, feat] = the mean.
  - PSUM -> SBUF copy (Act engine), grouped 8-tile output DMAs.

Measured on 8 axon trn2 cores: ~235 us device time (neuron-profile,
max over cores; baseline indirect-DMA version: ~843 us), rel err ~2.1e-3
vs the f32 reference (bf16 feature quantization).  Key limits: SWDGE
descriptor generation ~2.0 us per 1024-idx dma_gather instruction
(994 ns fixed + ~1 ns/desc, Pool-engine serial, 91 instructions) with
the HBM random-read drain (~20.5 ns/desc/engine) and PE matmuls
(~290 ns each, low p-state) overlapped underneath.

Staging columns are laid out chunk-major (so each <=8-unit gather
instruction writes consecutive columns); nid columns are laid out
tile-major (so each tile's B build reads consecutive columns).  The unit
structure depends on the indices, so the Bass program is built (and
compiled) per problem instance; SPMD across cores uses the max unit count
per (t, c) over the 8 cores (cores pad unused slots: idx 0, nid 255).
"""

import numpy as np
import ml_dtypes

import concourse.bass as bass
import concourse.bacc as bacc
import concourse.mybir as mybir
import concourse.tile as tile
from concourse import library_config
from concourse.bass_utils import run_bass_kernel_spmd

N = 100000
K = 6
V = 200000
D = 128
NCORES = 8
P = 128

NPC = N // NCORES            # 12500 nodes per core
TILES = -(-NPC // P)         # 98 node tiles per core
NPAD = TILES * P             # 12544
NCHUNK = 7
CH = -(-V // NCHUNK)         # 28572 rows per chunk (< 32768 for int16)
SB = 13                      # sub-batches (stages) per core: 7-8 tiles each,
                             # so each (sub-batch, chunk) is one <=1024-idx
                             # gather instruction and PE work bursts densely
MAXU = 8                     # units (128 idx each) per gather instruction
GROUP = 8                    # tiles per output DMA (>= tiles per sub-batch)
PADNID = 255.0
# feature dtype: bf16 on HW; f32 for CoreSim validation (sim lacks bf16
# dma_gather support)
FDT_M = mybir.dt.bfloat16
FDT_NP = ml_dtypes.bfloat16


def _split_tiles(tiles, sb):
    base, rem = divmod(tiles, sb)
    out, t0 = [], 0
    for s in range(sb):
        n = base + (1 if s < rem else 0)
        out.append((t0, t0 + n))
        t0 += n
    return out


def _balance(sig: np.ndarray, cap: int = P) -> np.ndarray | None:
    """Assign NPAD padded nodes (rows of sig: per-chunk entry counts; pads
    all-zero) to TILES groups of 128 so every per-group per-chunk count
    <= cap.  Returns pos[NPAD] (node -> slot; tile = pos//128) or None.

    Natural order start + swap repair: the expected overflow is a few
    dozen (t, c) cells a few entries over, with ~16% global slack.
    """
    npad, nch = sig.shape
    node_at = np.arange(npad)                 # slot -> node
    loads = sig.reshape(TILES, P, nch).sum(axis=1)    # [TILES, nch]
    for _ in range(4000):
        over = np.argwhere(loads > cap)
        if len(over) == 0:
            pos = np.empty(npad, np.int64)
            pos[node_at] = np.arange(npad)
            return pos
        g, c = over[np.argmax(loads[over[:, 0], over[:, 1]])]
        slots_g = np.arange(g * P, (g + 1) * P)
        sg = sig[node_at[slots_g]]            # [128, nch]
        n_local = int(np.argmax(sg[:, c]))
        n_slot = g * P + n_local
        sn = sig[node_at[n_slot]]
        # allowed post-swap load for g: strict progress on c, no new
        # overload beyond current level elsewhere
        glim = np.maximum(cap, loads[g])
        glim[c] = loads[g, c] - 1
        ok = False
        for t in np.argsort(loads[:, c]):
            if t == g:
                continue
            slots_t = np.arange(t * P, (t + 1) * P)
            st = sig[node_at[slots_t]]        # [128, nch]
            newg = loads[g][None, :] - sn[None, :] + st
            newt = loads[t][None, :] + sn[None, :] - st
            feas = ((newt <= cap).all(axis=1) & (newg <= glim).all(axis=1))
            if feas.any():
                cand = np.where(feas)[0]
                m_local = int(cand[np.argmin(st[cand, c])])
                m_slot = t * P + m_local
                sm = sig[node_at[m_slot]]
                loads[g] += sm - sn
                loads[t] += sn - sm
                node_at[n_slot], node_at[m_slot] = (
                    node_at[m_slot], node_at[n_slot])
                ok = True
                break
        if not ok:
            return None
    return None


class Plan:
    """Shared (SPMD) unit structure + per-core host arrays."""

    def __init__(self, neigh_idx: np.ndarray):
        ni = np.asarray(neigh_idx).astype(np.int64).reshape(NCORES, NPC, K)
        counts = np.zeros((NCORES, TILES, NCHUNK), np.int64)
        ents = []
        # pos of padded original node -> slot (tile = slot//128); balanced so
        # per-(tile, chunk) counts stay <= 128 (falls back to natural order
        # + multi-column units if repair fails)
        self.pos = np.zeros((NCORES, NPAD), np.int64)
        for core in range(NCORES):
            idx = ni[core].reshape(-1)              # NPC*K entries
            cc = idx // CH
            sig = np.zeros((NPAD, NCHUNK), np.int64)
            np.add.at(sig, (np.repeat(np.arange(NPC), K), cc), 1)
            pos = _balance(sig)
            if pos is None:
                pos = np.arange(NPAD)
            self.pos[core] = pos
            nn = np.repeat(pos[:NPC], K)            # slot of each entry
            tt = nn // P
            ll = (idx - cc * CH).astype(np.int16)
            key = tt * NCHUNK + cc
            counts[core] = np.bincount(
                key, minlength=TILES * NCHUNK).reshape(TILES, NCHUNK)
            ents.append((nn, tt, cc, ll, key))
        self.cols = -(-counts.max(axis=0) // P)      # [TILES, NCHUNK]
        assert (self.cols.sum(axis=1) > 0).all()
        self.maxj = int(self.cols.max())

        self.sb_ranges = _split_tiles(TILES, SB)
        self.nsb = len(self.sb_ranges)
        # chunk-major staging cols + tile-major nid cols, per sub-batch
        self.unit_col = {}       # (t,c,j) -> staging col within sb
        self.unit_tm = {}        # (t,c,j) -> tile-major col within sb
        self.tile_tmbase = {}    # t -> first tile-major col
        self.sb_ncols = []
        self.instr = [[] for _ in range(self.nsb)]    # (c, u0, take)
        self.idx_off = [[] for _ in range(self.nsb)]
        self.sb_idxcols = []
        for s, (ta, tb) in enumerate(self.sb_ranges):
            col = 0
            icols = 0
            for c in range(NCHUNK):
                cstart = col
                for t in range(ta, tb):
                    for j in range(self.cols[t, c]):
                        self.unit_col[(t, c, j)] = col
                        col += 1
                nu = col - cstart
                u0 = cstart
                while nu > 0:
                    take = min(MAXU, nu)
                    self.instr[s].append((c, u0, take))
                    self.idx_off[s].append(icols)
                    icols += take * P // 16
                    u0 += take
                    nu -= take
            tm = 0
            for t in range(ta, tb):
                self.tile_tmbase[t] = tm
                for c in range(NCHUNK):
                    for j in range(self.cols[t, c]):
                        self.unit_tm[(t, c, j)] = tm
                        tm += 1
            assert tm == col
            self.sb_ncols.append(col)
            self.sb_idxcols.append(icols)
        self.total_units = sum(self.sb_ncols)
        self.sb_colbase = np.concatenate(
            [[0], np.cumsum(self.sb_ncols)]).astype(np.int64)
        self.sb_idxbase = np.concatenate(
            [[0], np.cumsum(self.sb_idxcols)]).astype(np.int64)

        self.idx_dram = np.zeros((NCORES, P, int(self.sb_idxbase[-1])),
                                 np.int16)
        nid_f32 = np.full((NCORES, P, self.total_units), PADNID, np.float32)

        sbmap = np.zeros((TILES,), np.int64)
        for s, (ta, tb) in enumerate(self.sb_ranges):
            sbmap[ta:tb] = s
        MJ = self.maxj
        colmap = np.full((TILES * NCHUNK * MJ,), -1, np.int64)
        tmmap = np.full((TILES * NCHUNK * MJ,), -1, np.int64)
        for (t, c, j), col in self.unit_col.items():
            colmap[(t * NCHUNK + c) * MJ + j] = col
            tmmap[(t * NCHUNK + c) * MJ + j] = self.unit_tm[(t, c, j)]
        # per sb: staging col -> (idx dram col base, pos within instr)
        ucolbase = [np.zeros(n, np.int64) for n in self.sb_ncols]
        upos = [np.zeros(n, np.int64) for n in self.sb_ncols]
        for s in range(self.nsb):
            for (c, u0, take), ioff in zip(self.instr[s], self.idx_off[s]):
                for u in range(u0, u0 + take):
                    ucolbase[s][u] = self.sb_idxbase[s] + ioff
                    upos[s][u] = u - u0

        for core in range(NCORES):
            nn, tt, cc, ll, key = ents[core]
            # secondary sort by chunk-local row index: each unit's 128
            # descriptors read ascending HBM addresses
            order = np.lexsort((ll, key))
            skey = key[order]
            group_start = np.searchsorted(skey, np.arange(TILES * NCHUNK))
            rank = np.arange(len(order)) - group_start[skey]
            jj = rank // P
            pp = rank % P
            snn, stt, scc, sll = nn[order], tt[order], cc[order], ll[order]
            tcj = (stt * NCHUNK + scc) * MJ + jj
            ecol = colmap[tcj]
            etm = tmmap[tcj]
            assert (ecol >= 0).all()
            esb = sbmap[stt]
            gtm = self.sb_colbase[esb] + etm
            nid_f32[core, pp, gtm] = (snn % P).astype(np.float32)
            for s in range(self.nsb):
                m = esb == s
                if not m.any():
                    continue
                fl = upos[s][ecol[m]] * P + pp[m]
                icol = ucolbase[s][ecol[m]] + fl // 16
                irow = fl % 16
                for g in range(8):
                    self.idx_dram[core, irow + 16 * g, icol] = sll[m]
        self.nid_dram = nid_f32.astype(FDT_NP)

    def signature(self):
        return tuple(self.cols.reshape(-1).tolist())


def build_nc(plan: Plan, nq=4):
    nc = bacc.Bacc("TRN2", target_bir_lowering=False, num_swdge_queues=nq)
    featb = nc.dram_tensor("featb", [V, D], FDT_M,
                           kind="ExternalInput")
    idx = nc.dram_tensor("idx", [P, int(plan.sb_idxbase[-1])],
                         mybir.dt.int16, kind="ExternalInput")
    nid = nc.dram_tensor("nid", [P, int(plan.total_units)],
                         FDT_M, kind="ExternalInput")
    iota = nc.dram_tensor("iota", [P, P], FDT_M,
                          kind="ExternalInput")
    out = nc.dram_tensor("out", [P, TILES * D], mybir.dt.float32,
                         kind="ExternalOutput")
    with tile.TileContext(nc) as tc:
        nc.gpsimd.load_library(library_config.mlp)
        with tc.tile_pool(name="const", bufs=1) as constp, \
             tc.tile_pool(name="idxp", bufs=4) as idxp, \
             tc.tile_pool(name="nidp", bufs=4) as nidp, \
             tc.tile_pool(name="stg", bufs=4) as stgp, \
             tc.tile_pool(name="bp", bufs=3) as bp, \
             tc.tile_pool(name="accp", bufs=3) as accp, \
             tc.psum_pool(name="ps", bufs=4) as psp:
            iota_t = constp.tile([P, P], FDT_M)
            nc.sync.dma_start(out=iota_t[:], in_=iota[:])
            qi = 0
            for s in range(plan.nsb):
                ta, tb = plan.sb_ranges[s]
                ic0 = int(plan.sb_idxbase[s])
                icn = int(plan.sb_idxcols[s])
                cb = int(plan.sb_colbase[s])
                ncols = int(plan.sb_ncols[s])
                idx_t = idxp.tile([P, icn], mybir.dt.int16)
                nc.sync.dma_start(out=idx_t[:], in_=idx[:, ic0:ic0 + icn])
                nid_t = nidp.tile([P, ncols], FDT_M)
                nc.sync.dma_start(out=nid_t[:], in_=nid[:, cb:cb + ncols])
                stg = stgp.tile([P, ncols * D], FDT_M)
                for (c, u0, take), ioff in zip(plan.instr[s],
                                               plan.idx_off[s]):
                    ni_i = take * P
                    nc.gpsimd.dma_gather(
                        out_ap=stg[:, u0 * D:(u0 + take) * D].rearrange(
                            "p (u d) -> p u d", u=take, d=D),
                        in_ap=featb[c * CH:min((c + 1) * CH, V), :],
                        idxs_ap=idx_t[:, ioff:ioff + ni_i // 16],
                        num_idxs=ni_i,
                        num_idxs_reg=ni_i,
                        elem_size=D,
                        queue_num=qi % nq,
                    )
                    qi += 1
                acc = None
                for t in range(ta, tb):
                    ucols = [plan.unit_col[(t, c, j)]
                             for c in range(NCHUNK)
                             for j in range(plan.cols[t, c])]
                    nu = len(ucols)
                    tmb = plan.tile_tmbase[t]
                    gi_t = (t - ta) % GROUP
                    if gi_t == 0:
                        gsz = min(GROUP, tb - t)
                        acc = accp.tile([P, gsz * D], mybir.dt.float32)
                    bt = bp.tile([P, nu * P], FDT_M)
                    in0 = nid_t[:, tmb:tmb + nu].rearrange(
                        "p (u one) -> p u one", u=nu, one=1)
                    in1 = iota_t[:].rearrange(
                        "p (one d) -> p one d", one=1, d=P)
                    b0, b1 = bass.broadcast_tensor_aps(in0, in1)
                    nc.vector.tensor_tensor(
                        out=bt[:].rearrange("p (u d) -> p u d", u=nu, d=P),
                        in0=b0, in1=b1, op=mybir.AluOpType.is_equal)
                    ps = psp.tile([P, D], mybir.dt.float32)
                    for ui, ucol in enumerate(ucols):
                        nc.tensor.matmul(
                            ps[:],
                            lhsT=bt[:, ui * P:(ui + 1) * P],
                            rhs=stg[:, ucol * D:(ucol + 1) * D],
                            start=(ui == 0),
                            stop=(ui == nu - 1),
                        )
                    nc.scalar.copy(
                        out=acc[:, gi_t * D:(gi_t + 1) * D], in_=ps[:])
                    if gi_t == GROUP - 1 or t == tb - 1:
                        t0 = t - gi_t
                        nc.sync.dma_start(
                            out=out[:, t0 * D:(t + 1) * D],
                            in_=acc[:, :(gi_t + 1) * D])
    nc.compile()
    return nc


_cache = {}


def _get(plan):
    sig = plan.signature()
    if sig not in _cache:
        _cache[sig] = build_nc(plan)
    return _cache[sig]


def make_in_maps(features, neigh_idx, plan):
    feat = np.asarray(features, dtype=np.float32) * np.float32(1.0 / K)
    featb = feat.astype(FDT_NP)
    iota = np.ascontiguousarray(np.broadcast_to(
        np.arange(P, dtype=np.float32), (P, P))).astype(FDT_NP)
    return [{"featb": featb, "idx": np.ascontiguousarray(plan.idx_dram[c]),
             "nid": np.ascontiguousarray(plan.nid_dram[c]),
             "iota": np.ascontiguousarray(iota)}
            for c in range(NCORES)]


def assemble_out(results, plan):
    outs = []
    for c in range(NCORES):
        o = results[c]["out"]
        o = o.reshape(P, TILES, D).transpose(1, 0, 2).reshape(NPAD, D)
        outs.append(o[plan.pos[c, :NPC]])
    return np.ascontiguousarray(np.concatenate(outs, axis=0))


def kernel(features: np.ndarray, neigh_idx: np.ndarray, **run_kwargs):
    plan = Plan(neigh_idx)
    in_maps = make_in_maps(features, neigh_idx, plan)
    res = run_bass_kernel_spmd(_get(plan), in_maps,
                               core_ids=list(range(NCORES)), **run_kwargs)
    full = assemble_out(res.results, plan)
    if run_kwargs:
        return full, res
    return full
